# revision 30
# baseline (speedup 1.0000x reference)
"""MetaPathConnector kernel for Trainium2 (8 NeuronCores, Bass/Tile).

Row-shards N=16384 nodes across 8 cores (2048 rows each). Each core receives a
rotated copy of feat^T so that its own rows occupy columns [0, 2048) -- this
keeps the device program identical (static) on every core.

Per core:
  prep:  projT = W^T^T @ featT via 512-wide fp32r matmuls; one transpose round
         gives natural-layout proj (gather source -> DRAM) and row
         sum-of-squares (ACT square-accumulate from PSUM); rsqrt + transpose
         + flatten gives inv-norms indexed by column; a partition-broadcast
         stt column-scales projT in place into nrmT. No transpose-back round.
  main (per 128-row tile, fully interleaved):
         32 fp32r matmuls -> PSUM blocks [128,2048]; diagonal self-sim killed
         with a -8*I add; DVE max8/find_index8 -> per-block top-8 candidates
         + positions; top-10 refine + softmax; pack gather indices; one
         dma_gather of proj[idx]; GPSIMD accumulates w*proj + residual; DMA
         out. Gather/apply of tile t overlaps the scans of tile t+1.
"""

from contextlib import ExitStack

import numpy as np

import concourse.bass as bass
import concourse.mybir as mybir
import concourse.tile as tile
from concourse import bacc
from concourse.bass_utils import run_bass_kernel_spmd
from concourse.masks import make_identity

FP32 = mybir.dt.float32
FP32R = mybir.dt.float32r
U16 = mybir.dt.uint16
I16 = mybir.dt.int16
I32 = mybir.dt.int32
AF = mybir.ActivationFunctionType
ALU = mybir.AluOpType
AX = mybir.AxisListType

N_NODES = 16384
D = 128
N_CORES = 8
K = 10
STRENGTH = 0.1
BLK = 2048  # sims column block (4 PSUM banks of fp32)
MMW = 512  # fp32 matmul moving-operand max
SENTINEL = -8.0


def build_nc(n_nodes=N_NODES, rows=N_NODES // N_CORES, n_cores=N_CORES,
             debug=False, stage=5, reps=1):
    nc = bacc.Bacc("TRN2", target_bir_lowering=False, debug=debug,
                   num_devices=n_cores)
    featT = nc.dram_tensor("featT", [D, n_nodes], FP32, kind="ExternalInput")
    feat_rows = nc.dram_tensor("feat_rows", [rows, D], FP32,
                               kind="ExternalInput")
    WT = nc.dram_tensor("WT", [D, D], FP32, kind="ExternalInput")
    emb = nc.dram_tensor("emb", [1, D], FP32, kind="ExternalInput")
    out_rows = nc.dram_tensor("out_rows", [rows, D], FP32,
                              kind="ExternalOutput")
    projdram = nc.dram_tensor("projdram", [n_nodes, D], FP32)
    invdram = nc.dram_tensor("invdram", [1, n_nodes], FP32)

    with tile.TileContext(nc) as tc, ExitStack() as ctx:
        _build(ctx, tc, featT.ap(), feat_rows.ap(), WT.ap(), emb.ap(),
               out_rows.ap(), projdram.ap(), invdram.ap(), n_nodes, rows,
               stage, reps)
    nc.compile()
    return nc


def _build(ctx, tc, featT, feat_rows, WT, emb, out_rows, projdram, invdram,
           n_nodes, rows, stage=5, reps=1):
    nc = tc.nc
    n_blocks = n_nodes // BLK          # 8
    n_tiles = rows // 128              # 16
    CW = n_blocks * 8                  # candidate width per row (64)

    consts = ctx.enter_context(tc.tile_pool(name="consts", bufs=1))
    bigbuf = ctx.enter_context(tc.tile_pool(name="bigbuf", bufs=1))
    stream = ctx.enter_context(tc.tile_pool(name="stream", bufs=2))
    small = ctx.enter_context(tc.tile_pool(name="small", bufs=3))
    tpool = ctx.enter_context(tc.tile_pool(name="tpool", bufs=4))
    gpool = ctx.enter_context(tc.tile_pool(name="gpool", bufs=8))
    psum_blk = ctx.enter_context(
        tc.tile_pool(name="psum_blk", bufs=2, space="PSUM"))

    # ---------------- constants ----------------
    ident = consts.tile([128, 128], FP32)
    make_identity(nc, ident[:])
    negI = consts.tile([128, 128], FP32)
    nc.gpsimd.memset(negI[:], 0.0)
    nc.gpsimd.affine_select(
        out=negI[:], in_=negI[:], compare_op=ALU.not_equal, fill=SENTINEL,
        base=0, pattern=[[-1, 128]], channel_multiplier=1)

    iota_i = consts.tile([128, CW], I32)
    nc.gpsimd.iota(iota_i[:], pattern=[[1, CW]], base=0, channel_multiplier=0)
    iotaF = consts.tile([128, CW], FP32)
    nc.vector.tensor_copy(iotaF[:], iota_i[:])

    bb_i = consts.tile([128, CW], I32)
    nc.gpsimd.iota(bb_i[:], pattern=[[BLK, n_blocks], [0, 8]], base=0,
                   channel_multiplier=0)
    bbF = consts.tile([128, CW], FP32)
    nc.vector.tensor_copy(bbF[:], bb_i[:])

    emb_bc = consts.tile([128, D], FP32)
    nc.sync.dma_start(out=emb_bc[:], in_=emb.to_broadcast([128, D]))
    emb01 = consts.tile([128, D], FP32)
    nc.scalar.mul(emb01[:], emb_bc[:], STRENGTH)

    WT_sb = consts.tile([D, D], FP32)
    nc.sync.dma_start(out=WT_sb[:], in_=WT)

    def _body():
        # ------- prep: projT -> pnat + norms -> inv-flat -> nrmT (in place) ---
        nrmT = bigbuf.tile([128, n_nodes], FP32)   # holds projT, then nrmT
        nchunks = n_nodes // 128              # 128
        ngrp = nchunks // 4                   # 512-wide groups (32)
        ssq_all = consts.tile([128, nchunks], FP32)

        for b in range(n_blocks):
            fT = stream.tile([128, BLK], FP32, tag="ftblk")
            nc.sync.dma_start(out=fT[:], in_=featT[:, b * BLK:(b + 1) * BLK])
            pA = psum_blk.tile([128, BLK], FP32, tag="blk")
            for m in range(BLK // MMW):
                nc.tensor.matmul(pA[:, m * MMW:(m + 1) * MMW], lhsT=WT_sb[:],
                                 rhs=fT[:, m * MMW:(m + 1) * MMW], start=True,
                                 stop=True)
            if b % 2 == 0:
                nc.scalar.copy(nrmT[:, b * BLK:(b + 1) * BLK], pA[:])
            else:
                nc.vector.tensor_copy(nrmT[:, b * BLK:(b + 1) * BLK], pA[:])

        sqscr = consts.tile([128, 512], FP32)
        for g in range(ngrp):
            pT = psum_blk.tile([128, BLK], FP32, tag="blk")
            pG = pT[:, 0:512]
            for j in range(4):
                c = 4 * g + j
                nc.tensor.transpose(pG[:, 128 * j:128 * (j + 1)],
                                    nrmT[:, c * 128:(c + 1) * 128], ident[:])
            pgrp = stream.tile([128, 512], FP32, tag="pgrp")
            nc.scalar.copy(pgrp[:], pG)
            sq = stream.tile([128, 512], FP32, tag="sqg")
            nc.scalar.activation(sq[:], pG, AF.Square)
            nc.vector.tensor_reduce(
                ssq_all[:, 4 * g:4 * (g + 1)],
                sq[:].rearrange("p (c d) -> p c d", d=128),
                axis=AX.X, op=ALU.add)
            # proj rows for this group -> DRAM (gather source), unnormalized
            nc.sync.dma_start(
                out=projdram[4 * g * 128:(4 * g + 4) * 128, :].rearrange(
                    "(c p) d -> p c d", p=128),
                in_=pgrp[:].rearrange("p (c d) -> p c d", d=128))
        # inv-norms, indexed by node: invT[c, p] = 1/nrm(node c*128+p)
        nrm_all = consts.tile([128, nchunks], FP32)
        nc.scalar.sqrt(nrm_all[:], ssq_all[:])
        inv_all = consts.tile([128, nchunks], FP32)
        nc.vector.reciprocal(inv_all[:], nrm_all[:])
        pT = psum_blk.tile([128, BLK], FP32, tag="blk")
        nc.tensor.transpose(pT[:, 0:128], inv_all[:], ident[:])
        invT = consts.tile([128, nchunks], FP32)
        nc.scalar.copy(invT[:], pT[:, 0:128])
        nc.sync.dma_start(
            out=invdram.rearrange("q (c p) -> q c p", p=128),
            in_=invT[:].rearrange("c (q p) -> c q p", q=1))
        inv_bc = bigbuf.tile([128, n_nodes], FP32)
        # column-scale projT in place -> nrmT (broadcast DMA split + interleaved)
        SC = 4096
        for s in range(n_nodes // SC):
            nc.sync.dma_start(
                out=inv_bc[:, s * SC:(s + 1) * SC],
                in_=invdram[:, s * SC:(s + 1) * SC].to_broadcast([128, SC]))
        for s in range(n_nodes // SC):
            nc.vector.scalar_tensor_tensor(
                out=nrmT[:, s * SC:(s + 1) * SC],
                in0=nrmT[:, s * SC:(s + 1) * SC], scalar=1.0,
                in1=inv_bc[:, s * SC:(s + 1) * SC],
                op0=ALU.mult, op1=ALU.mult)

        def _bail():
            for t in range(n_tiles):
                ft = small.tile([128, D], FP32, tag="ft")
                nc.sync.dma_start(out=ft[:],
                                  in_=feat_rows[t * 128:(t + 1) * 128, :])
                o = small.tile([128, D], FP32, tag="oo")
                nc.vector.tensor_add(o[:], ft[:], nrmT[:, t * 128:(t + 1) * 128])
                nc.sync.dma_start(out=out_rows[t * 128:(t + 1) * 128, :], in_=o[:])

        if stage <= 1:
            _bail()
            return

        # ------- main: per tile: sims + candidates + topk + gather launch;
        # the weighted apply of tile t-1 is issued after tile t's scans so the
        # gather DMA never stalls the DVE stream. -------
        def _front(t):
            lhs = nrmT[:, t * 128:(t + 1) * 128]
            cand = tpool.tile([128, CW], FP32, tag="cand")
            cidx = tpool.tile([128, CW], U16, tag="cidx")
            for b in range(n_blocks):
                ps = psum_blk.tile([128, BLK], FP32, tag="blk")
                for m in range(BLK // MMW):
                    nc.tensor.matmul(
                        ps[:, m * MMW:(m + 1) * MMW], lhsT=lhs,
                        rhs=nrmT[:, b * BLK + m * MMW:b * BLK + (m + 1) * MMW],
                        start=True, stop=True)
                if b == (t * 128) // BLK:
                    off = (t * 128) % BLK
                    nc.vector.tensor_add(ps[:, off:off + 128],
                                         ps[:, off:off + 128], negI[:])
                nc.vector.max(out=cand[:, b * 8:(b + 1) * 8], in_=ps[:])
                nc.vector.max_index(out=cidx[:, b * 8:(b + 1) * 8],
                                    in_max=cand[:, b * 8:(b + 1) * 8],
                                    in_values=ps[:])
            if stage <= 2:
                return None
            # global candidate columns (fp32 exact): ACT cast issued early,
            # DVE add deferred to just before its use (one-hot loop)
            cgid = tpool.tile([128, CW], FP32, tag="cgid")
            nc.scalar.copy(cgid[:], cidx[:])

            top8a = tpool.tile([128, 8], FP32, tag="t8a")
            nc.vector.max(out=top8a[:], in_=cand[:])
            cand2 = tpool.tile([128, CW], FP32, tag="cand2")
            nc.vector.match_replace(out=cand2[:], in_to_replace=top8a[:],
                                    in_values=cand[:], imm_value=SENTINEL)
            top8b = tpool.tile([128, 8], FP32, tag="t8b")
            nc.vector.max(out=top8b[:], in_=cand2[:])
            posa = tpool.tile([128, 8], U16, tag="posa")
            nc.vector.max_index(out=posa[:], in_max=top8a[:], in_values=cand[:])
            posb = tpool.tile([128, 8], U16, tag="posb")
            nc.vector.max_index(out=posb[:], in_max=top8b[:], in_values=cand2[:])

            P16 = tpool.tile([128, 16], FP32, tag="p16")
            nc.scalar.copy(P16[:, 0:8], posa[:])
            nc.scalar.copy(P16[:, 8:16], posb[:])

            # exp of top-10 values; Z; 0.1/Z
            V10 = tpool.tile([128, K], FP32, tag="v10")
            nc.scalar.copy(V10[:, 0:8], top8a[:])
            nc.scalar.copy(V10[:, 8:K], top8b[:, 0:K - 8])
            E10 = tpool.tile([128, K], FP32, tag="e10")
            nc.scalar.activation(E10[:], V10[:], AF.Exp)
            Z = tpool.tile([128, 1], FP32, tag="zz")
            nc.vector.tensor_reduce(Z[:], E10[:], axis=AX.X, op=ALU.add)
            invZ01 = tpool.tile([128, 1], FP32, tag="iz")
            nc.vector.reciprocal(invZ01[:], Z[:])
            nc.vector.tensor_scalar_mul(invZ01[:], invZ01[:], STRENGTH)

            nc.vector.tensor_add(cgid[:], cgid[:], bbF[:])
            # per-row global columns of the top-10 (one-hot dot)
            gf = tpool.tile([128, K], FP32, tag="gf")
            for k in range(K):
                scr = small.tile([128, CW], FP32, tag="scr")
                nc.vector.scalar_tensor_tensor(
                    out=scr[:], in0=iotaF[:], scalar=P16[:, k:k + 1], in1=cgid[:],
                    op0=ALU.is_equal, op1=ALU.mult,
                    accum_out=gf[:, k:k + 1])
            gidx16 = tpool.tile([128, K], I16, tag="gidx")
            nc.vector.tensor_copy(gidx16[:], gf[:])
            if stage <= 3:
                return None

            # pack gather indices: idx i at [i % 16, i // 16] replicated
            # across all eight 16-partition groups; i = j*128 + p.
            idxw = tpool.tile([128, K * 8], I16, tag="idxw")
            for a in range(8):
                nc.sync.dma_start(
                    out=idxw[0:16, a:K * 8:8],
                    in_=gidx16[16 * a:16 * (a + 1), :])
            for q in range(1, 8):
                nc.sync.dma_start(out=idxw[16 * q:16 * (q + 1), :],
                                  in_=idxw[0:16, :])
            GCOLS = 5
            Gs = []
            for h in range(2):
                G = gpool.tile([128, GCOLS, D], FP32, tag="gath")
                nc.gpsimd.dma_gather(
                    out_ap=G[:], in_ap=projdram,
                    idxs_ap=idxw[:, h * GCOLS * 8:(h + 1) * GCOLS * 8],
                    num_idxs=GCOLS * 128, num_idxs_reg=GCOLS * 128,
                    elem_size=D, queue_num=0)
                Gs.append(G)
            if stage <= 4:
                return None
            return (t, Gs, E10, invZ01)

        def _apply(state):
            t, Gs, E10, invZ01 = state
            acc = tpool.tile([128, D], FP32, tag="acc")
            nc.vector.memset(acc[:], 0.0)
            for k in range(0, K):
                nc.vector.scalar_tensor_tensor(
                    out=acc[:], in0=Gs[k // 5][:, k % 5, :].squeeze(),
                    scalar=E10[:, k:k + 1], in1=acc[:],
                    op0=ALU.mult, op1=ALU.add)
            ft = tpool.tile([128, D], FP32, tag="ft")
            nc.sync.dma_start(out=ft[:],
                              in_=feat_rows[t * 128:(t + 1) * 128, :])
            o = tpool.tile([128, D], FP32, tag="oo")
            nc.vector.scalar_tensor_tensor(
                out=o[:], in0=acc[:], scalar=invZ01[:], in1=ft[:],
                op0=ALU.mult, op1=ALU.add)
            nc.vector.tensor_add(o[:], o[:], emb01[:])
            nc.sync.dma_start(out=out_rows[t * 128:(t + 1) * 128, :],
                              in_=o[:])

        from collections import deque
        pending = deque()
        for t in range(n_tiles):
            if stage <= 2 and t > 0:
                break
            st = _front(t)
            if st is not None:
                pending.append(st)
            while len(pending) > 2:
                _apply(pending.popleft())
        while pending:
            _apply(pending.popleft())

    if reps == 1:
        _body()
    else:
        with tc.For_i(0, reps, 1):
            _body()


_NC_CACHE = {}


def _get_nc(n_nodes, rows, n_cores):
    key = (n_nodes, rows, n_cores)
    if key not in _NC_CACHE:
        _NC_CACHE[key] = build_nc(n_nodes, rows, n_cores)
    return _NC_CACHE[key]


def make_in_maps(feat, W, emb, n_cores=N_CORES):
    n = feat.shape[0]
    rows = n // n_cores
    featT = np.ascontiguousarray(feat.T.astype(np.float32))
    WT = np.ascontiguousarray(W.T.astype(np.float32))
    emb = np.ascontiguousarray(emb.astype(np.float32))
    maps = []
    for c in range(n_cores):
        maps.append({
            "featT": np.ascontiguousarray(np.roll(featT, -rows * c, axis=1)),
            "feat_rows": np.ascontiguousarray(feat[rows * c:rows * (c + 1)]),
            "WT": WT,
            "emb": emb,
        })
    return maps


def kernel(feat, W, emb):
    feat = np.asarray(feat, dtype=np.float32)
    W = np.asarray(W, dtype=np.float32)
    emb = np.asarray(emb, dtype=np.float32)
    n = feat.shape[0]
    rows = n // N_CORES
    nc = _get_nc(n, rows, N_CORES)
    in_maps = make_in_maps(feat, W, emb, N_CORES)
    res = run_bass_kernel_spmd(nc, in_maps, core_ids=list(range(N_CORES)))
    out = np.concatenate([res.results[c]["out_rows"] for c in range(N_CORES)],
                         axis=0)
    return out.astype(np.float32)


# revision 31
# speedup vs baseline: 1.0215x; 1.0215x over previous
"""MetaPathConnector kernel for Trainium2 (8 NeuronCores, Bass/Tile).

Row-shards N=16384 nodes across 8 cores (2048 rows each). Each core receives a
rotated copy of feat^T so that its own rows occupy columns [0, 2048) -- this
keeps the device program identical (static) on every core.

Per core:
  prep:  projT = W^T^T @ featT via 512-wide fp32r matmuls; one transpose round
         gives natural-layout proj (gather source -> DRAM) and row
         sum-of-squares (ACT square-accumulate from PSUM); rsqrt + transpose
         + flatten gives inv-norms indexed by column; a partition-broadcast
         stt column-scales projT in place into nrmT. No transpose-back round.
  main (per 128-row tile, fully interleaved):
         32 fp32r matmuls -> PSUM blocks [128,2048]; diagonal self-sim killed
         with a -8*I add; DVE max8/find_index8 -> per-block top-8 candidates
         + positions; top-10 refine + softmax; pack gather indices; one
         dma_gather of proj[idx]; GPSIMD accumulates w*proj + residual; DMA
         out. Gather/apply of tile t overlaps the scans of tile t+1.
"""

from contextlib import ExitStack

import numpy as np

import concourse.bass as bass
import concourse.mybir as mybir
import concourse.tile as tile
from concourse import bacc
from concourse.bass_utils import run_bass_kernel_spmd
from concourse.masks import make_identity

FP32 = mybir.dt.float32
FP32R = mybir.dt.float32r
U16 = mybir.dt.uint16
I16 = mybir.dt.int16
I32 = mybir.dt.int32
AF = mybir.ActivationFunctionType
ALU = mybir.AluOpType
AX = mybir.AxisListType

N_NODES = 16384
D = 128
N_CORES = 8
K = 10
STRENGTH = 0.1
BLK = 2048  # sims column block (4 PSUM banks of fp32)
MMW = 512  # fp32 matmul moving-operand max
SENTINEL = -8.0


def build_nc(n_nodes=N_NODES, rows=N_NODES // N_CORES, n_cores=N_CORES,
             debug=False, stage=5, reps=1):
    nc = bacc.Bacc("TRN2", target_bir_lowering=False, debug=debug,
                   num_devices=n_cores)
    featT = nc.dram_tensor("featT", [D, n_nodes], FP32, kind="ExternalInput")
    feat_rows = nc.dram_tensor("feat_rows", [rows, D], FP32,
                               kind="ExternalInput")
    WT = nc.dram_tensor("WT", [D, D], FP32, kind="ExternalInput")
    emb = nc.dram_tensor("emb", [1, D], FP32, kind="ExternalInput")
    out_rows = nc.dram_tensor("out_rows", [rows, D], FP32,
                              kind="ExternalOutput")
    projdram = nc.dram_tensor("projdram", [n_nodes, D], FP32)
    invdram = nc.dram_tensor("invdram", [1, n_nodes], FP32)

    with tile.TileContext(nc) as tc, ExitStack() as ctx:
        _build(ctx, tc, featT.ap(), feat_rows.ap(), WT.ap(), emb.ap(),
               out_rows.ap(), projdram.ap(), invdram.ap(), n_nodes, rows,
               stage, reps)
    nc.compile()
    return nc


def _build(ctx, tc, featT, feat_rows, WT, emb, out_rows, projdram, invdram,
           n_nodes, rows, stage=5, reps=1):
    nc = tc.nc
    n_blocks = n_nodes // BLK          # 8
    n_tiles = rows // 128              # 16
    CW = n_blocks * 8                  # candidate width per row (64)

    consts = ctx.enter_context(tc.tile_pool(name="consts", bufs=1))
    bigbuf = ctx.enter_context(tc.tile_pool(name="bigbuf", bufs=1))
    stream = ctx.enter_context(tc.tile_pool(name="stream", bufs=2))
    small = ctx.enter_context(tc.tile_pool(name="small", bufs=3))
    tpool = ctx.enter_context(tc.tile_pool(name="tpool", bufs=4))
    gpool = ctx.enter_context(tc.tile_pool(name="gpool", bufs=8))
    psum_blk = ctx.enter_context(
        tc.tile_pool(name="psum_blk", bufs=2, space="PSUM"))

    # ---------------- constants ----------------
    ident = consts.tile([128, 128], FP32)
    make_identity(nc, ident[:])
    negI = consts.tile([128, 128], FP32)
    nc.gpsimd.memset(negI[:], 0.0)
    nc.gpsimd.affine_select(
        out=negI[:], in_=negI[:], compare_op=ALU.not_equal, fill=SENTINEL,
        base=0, pattern=[[-1, 128]], channel_multiplier=1)

    iota_i = consts.tile([128, CW], I32)
    nc.gpsimd.iota(iota_i[:], pattern=[[1, CW]], base=0, channel_multiplier=0)
    iotaF = consts.tile([128, CW], FP32)
    nc.vector.tensor_copy(iotaF[:], iota_i[:])

    bb_i = consts.tile([128, CW], I32)
    nc.gpsimd.iota(bb_i[:], pattern=[[BLK, n_blocks], [0, 8]], base=0,
                   channel_multiplier=0)
    bbF = consts.tile([128, CW], FP32)
    nc.vector.tensor_copy(bbF[:], bb_i[:])

    emb_bc = consts.tile([128, D], FP32)
    nc.sync.dma_start(out=emb_bc[:], in_=emb.to_broadcast([128, D]))
    emb01 = consts.tile([128, D], FP32)
    nc.scalar.mul(emb01[:], emb_bc[:], STRENGTH)

    WT_sb = consts.tile([D, D], FP32)
    nc.sync.dma_start(out=WT_sb[:], in_=WT)

    def _body():
        # ------- prep: projT -> pnat + norms -> inv-flat -> nrmT (in place) ---
        nrmT = bigbuf.tile([128, n_nodes], FP32)   # holds projT, then nrmT
        nchunks = n_nodes // 128              # 128
        ngrp = nchunks // 4                   # 512-wide groups (32)
        ssq_all = consts.tile([128, nchunks], FP32)

        for b in range(n_blocks):
            fT = stream.tile([128, BLK], FP32, tag="ftblk")
            nc.sync.dma_start(out=fT[:], in_=featT[:, b * BLK:(b + 1) * BLK])
            pA = psum_blk.tile([128, BLK], FP32, tag="blk")
            for m in range(BLK // MMW):
                nc.tensor.matmul(pA[:, m * MMW:(m + 1) * MMW], lhsT=WT_sb[:],
                                 rhs=fT[:, m * MMW:(m + 1) * MMW], start=True,
                                 stop=True)
            if b % 2 == 0:
                nc.scalar.copy(nrmT[:, b * BLK:(b + 1) * BLK], pA[:])
            else:
                nc.vector.tensor_copy(nrmT[:, b * BLK:(b + 1) * BLK], pA[:])

        sqscr = consts.tile([128, 512], FP32)
        for g in range(ngrp):
            pT = psum_blk.tile([128, BLK], FP32, tag="blk")
            pG = pT[:, 0:512]
            for j in range(4):
                c = 4 * g + j
                nc.tensor.transpose(pG[:, 128 * j:128 * (j + 1)],
                                    nrmT[:, c * 128:(c + 1) * 128], ident[:])
            pgrp = stream.tile([128, 512], FP32, tag="pgrp")
            nc.scalar.copy(pgrp[:], pG)
            sq = stream.tile([128, 512], FP32, tag="sqg")
            nc.scalar.activation(sq[:], pG, AF.Square)
            nc.vector.tensor_reduce(
                ssq_all[:, 4 * g:4 * (g + 1)],
                sq[:].rearrange("p (c d) -> p c d", d=128),
                axis=AX.X, op=ALU.add)
            # proj rows for this group -> DRAM (gather source), unnormalized
            nc.sync.dma_start(
                out=projdram[4 * g * 128:(4 * g + 4) * 128, :].rearrange(
                    "(c p) d -> p c d", p=128),
                in_=pgrp[:].rearrange("p (c d) -> p c d", d=128))
        # inv-norms, indexed by node: invT[c, p] = 1/nrm(node c*128+p)
        nrm_all = consts.tile([128, nchunks], FP32)
        nc.scalar.sqrt(nrm_all[:], ssq_all[:])
        inv_all = consts.tile([128, nchunks], FP32)
        nc.vector.reciprocal(inv_all[:], nrm_all[:])
        pT = psum_blk.tile([128, BLK], FP32, tag="blk")
        nc.tensor.transpose(pT[:, 0:128], inv_all[:], ident[:])
        invT = consts.tile([128, nchunks], FP32)
        nc.scalar.copy(invT[:], pT[:, 0:128])
        nc.sync.dma_start(
            out=invdram.rearrange("q (c p) -> q c p", p=128),
            in_=invT[:].rearrange("c (q p) -> c q p", q=1))
        inv_bc = bigbuf.tile([128, n_nodes], FP32)
        # column-scale projT in place -> nrmT (broadcast DMA split + interleaved)
        SC = 4096
        for s in range(n_nodes // SC):
            nc.sync.dma_start(
                out=inv_bc[:, s * SC:(s + 1) * SC],
                in_=invdram[:, s * SC:(s + 1) * SC].to_broadcast([128, SC]))
        for s in range(n_nodes // SC):
            nc.vector.scalar_tensor_tensor(
                out=nrmT[:, s * SC:(s + 1) * SC],
                in0=nrmT[:, s * SC:(s + 1) * SC], scalar=1.0,
                in1=inv_bc[:, s * SC:(s + 1) * SC],
                op0=ALU.mult, op1=ALU.mult)

        def _bail():
            for t in range(n_tiles):
                ft = small.tile([128, D], FP32, tag="ft")
                nc.sync.dma_start(out=ft[:],
                                  in_=feat_rows[t * 128:(t + 1) * 128, :])
                o = small.tile([128, D], FP32, tag="oo")
                nc.vector.tensor_add(o[:], ft[:], nrmT[:, t * 128:(t + 1) * 128])
                nc.sync.dma_start(out=out_rows[t * 128:(t + 1) * 128, :], in_=o[:])

        if stage <= 1:
            _bail()
            return

        # ------- main: per tile: sims + candidates + topk + gather launch;
        # the weighted apply of tile t-1 is issued after tile t's scans so the
        # gather DMA never stalls the DVE stream. -------
        def _front(t):
            lhs = nrmT[:, t * 128:(t + 1) * 128]
            cand = tpool.tile([128, CW], FP32, tag="cand")
            cidx = tpool.tile([128, CW], U16, tag="cidx")
            for b in range(n_blocks):
                ps = psum_blk.tile([128, BLK], FP32, tag="blk")
                for m in range(BLK // MMW):
                    nc.tensor.matmul(
                        ps[:, m * MMW:(m + 1) * MMW], lhsT=lhs,
                        rhs=nrmT[:, b * BLK + m * MMW:b * BLK + (m + 1) * MMW],
                        start=True, stop=True)
                if b == (t * 128) // BLK:
                    off = (t * 128) % BLK
                    nc.vector.tensor_add(ps[:, off:off + 128],
                                         ps[:, off:off + 128], negI[:])
                nc.vector.max(out=cand[:, b * 8:(b + 1) * 8], in_=ps[:])
                nc.vector.max_index(out=cidx[:, b * 8:(b + 1) * 8],
                                    in_max=cand[:, b * 8:(b + 1) * 8],
                                    in_values=ps[:])
            if stage <= 2:
                return None
            # global candidate columns (fp32 exact)
            cgid = tpool.tile([128, CW], FP32, tag="cgid")
            nc.scalar.copy(cgid[:], cidx[:])
            nc.vector.tensor_add(cgid[:], cgid[:], bbF[:])

            top8a = tpool.tile([128, 8], FP32, tag="t8a")
            nc.vector.max(out=top8a[:], in_=cand[:])
            cand2 = tpool.tile([128, CW], FP32, tag="cand2")
            nc.vector.match_replace(out=cand2[:], in_to_replace=top8a[:],
                                    in_values=cand[:], imm_value=SENTINEL)
            top8b = tpool.tile([128, 8], FP32, tag="t8b")
            nc.vector.max(out=top8b[:], in_=cand2[:])
            posa = tpool.tile([128, 8], U16, tag="posa")
            nc.vector.max_index(out=posa[:], in_max=top8a[:], in_values=cand[:])
            posb = tpool.tile([128, 8], U16, tag="posb")
            nc.vector.max_index(out=posb[:], in_max=top8b[:], in_values=cand2[:])

            P16 = tpool.tile([128, 16], FP32, tag="p16")
            nc.scalar.copy(P16[:, 0:8], posa[:])
            nc.scalar.copy(P16[:, 8:16], posb[:])

            # exp of top-10 values; Z; 0.1/Z
            V10 = tpool.tile([128, K], FP32, tag="v10")
            nc.scalar.copy(V10[:, 0:8], top8a[:])
            nc.scalar.copy(V10[:, 8:K], top8b[:, 0:K - 8])
            E10 = tpool.tile([128, K], FP32, tag="e10")
            nc.scalar.activation(E10[:], V10[:], AF.Exp)
            Z = tpool.tile([128, 1], FP32, tag="zz")
            nc.vector.tensor_reduce(Z[:], E10[:], axis=AX.X, op=ALU.add)
            invZ01 = tpool.tile([128, 1], FP32, tag="iz")
            nc.vector.reciprocal(invZ01[:], Z[:])
            nc.vector.tensor_scalar_mul(invZ01[:], invZ01[:], STRENGTH)

            # per-row global columns of the top-10 (one-hot dot)
            gf = tpool.tile([128, K], FP32, tag="gf")
            for k in range(K):
                scr = small.tile([128, CW], FP32, tag="scr")
                nc.vector.scalar_tensor_tensor(
                    out=scr[:], in0=iotaF[:], scalar=P16[:, k:k + 1], in1=cgid[:],
                    op0=ALU.is_equal, op1=ALU.mult,
                    accum_out=gf[:, k:k + 1])
            gidx16 = tpool.tile([128, K], I16, tag="gidx")
            nc.vector.tensor_copy(gidx16[:], gf[:])
            if stage <= 3:
                return None

            # pack gather indices: idx i at [i % 16, i // 16] replicated
            # across all eight 16-partition groups; i = j*128 + p.
            idxw = tpool.tile([128, K * 8], I16, tag="idxw")
            for a in range(8):
                nc.sync.dma_start(
                    out=idxw[0:16, a:K * 8:8],
                    in_=gidx16[16 * a:16 * (a + 1), :])
            for q in range(1, 8):
                nc.sync.dma_start(out=idxw[16 * q:16 * (q + 1), :],
                                  in_=idxw[0:16, :])
            GCOLS = 5
            Gs = []
            for h in range(2):
                G = gpool.tile([128, GCOLS, D], FP32, tag="gath")
                nc.gpsimd.dma_gather(
                    out_ap=G[:], in_ap=projdram,
                    idxs_ap=idxw[:, h * GCOLS * 8:(h + 1) * GCOLS * 8],
                    num_idxs=GCOLS * 128, num_idxs_reg=GCOLS * 128,
                    elem_size=D, queue_num=0)
                Gs.append(G)
            if stage <= 4:
                return None
            return (t, Gs, E10, invZ01)

        def _apply(state):
            t, Gs, E10, invZ01 = state
            acc = tpool.tile([128, D], FP32, tag="acc")
            nc.vector.memset(acc[:], 0.0)
            for k in range(0, K):
                nc.vector.scalar_tensor_tensor(
                    out=acc[:], in0=Gs[k // 5][:, k % 5, :].squeeze(),
                    scalar=E10[:, k:k + 1], in1=acc[:],
                    op0=ALU.mult, op1=ALU.add)
            ft = tpool.tile([128, D], FP32, tag="ft")
            nc.sync.dma_start(out=ft[:],
                              in_=feat_rows[t * 128:(t + 1) * 128, :])
            o = tpool.tile([128, D], FP32, tag="oo")
            nc.vector.scalar_tensor_tensor(
                out=o[:], in0=acc[:], scalar=invZ01[:], in1=ft[:],
                op0=ALU.mult, op1=ALU.add)
            nc.vector.tensor_add(o[:], o[:], emb01[:])
            nc.sync.dma_start(out=out_rows[t * 128:(t + 1) * 128, :],
                              in_=o[:])

        from collections import deque
        pending = deque()
        for t in range(n_tiles):
            if stage <= 2 and t > 0:
                break
            st = _front(t)
            if st is not None:
                pending.append(st)
            while len(pending) > 2:
                _apply(pending.popleft())
        while pending:
            _apply(pending.popleft())

    if reps == 1:
        _body()
    else:
        with tc.For_i(0, reps, 1):
            _body()


_NC_CACHE = {}


def _get_nc(n_nodes, rows, n_cores):
    key = (n_nodes, rows, n_cores)
    if key not in _NC_CACHE:
        _NC_CACHE[key] = build_nc(n_nodes, rows, n_cores)
    return _NC_CACHE[key]


def make_in_maps(feat, W, emb, n_cores=N_CORES):
    n = feat.shape[0]
    rows = n // n_cores
    featT = np.ascontiguousarray(feat.T.astype(np.float32))
    WT = np.ascontiguousarray(W.T.astype(np.float32))
    emb = np.ascontiguousarray(emb.astype(np.float32))
    maps = []
    for c in range(n_cores):
        maps.append({
            "featT": np.ascontiguousarray(np.roll(featT, -rows * c, axis=1)),
            "feat_rows": np.ascontiguousarray(feat[rows * c:rows * (c + 1)]),
            "WT": WT,
            "emb": emb,
        })
    return maps


def kernel(feat, W, emb):
    feat = np.asarray(feat, dtype=np.float32)
    W = np.asarray(W, dtype=np.float32)
    emb = np.asarray(emb, dtype=np.float32)
    n = feat.shape[0]
    rows = n // N_CORES
    nc = _get_nc(n, rows, N_CORES)
    in_maps = make_in_maps(feat, W, emb, N_CORES)
    res = run_bass_kernel_spmd(nc, in_maps, core_ids=list(range(N_CORES)))
    out = np.concatenate([res.results[c]["out_rows"] for c in range(N_CORES)],
                         axis=0)
    return out.astype(np.float32)


# revision 32
# speedup vs baseline: 1.0442x; 1.0222x over previous
"""MetaPathConnector kernel for Trainium2 (8 NeuronCores, Bass/Tile).

Row-shards N=16384 nodes across 8 cores (2048 rows each). Each core receives a
rotated copy of feat^T so that its own rows occupy columns [0, 2048) -- this
keeps the device program identical (static) on every core.

Per core:
  prep:  projT = W^T^T @ featT via 512-wide fp32r matmuls; one transpose round
         gives natural-layout proj (gather source -> DRAM) and row
         sum-of-squares (ACT square-accumulate from PSUM); rsqrt + transpose
         + flatten gives inv-norms indexed by column; a partition-broadcast
         stt column-scales projT in place into nrmT. No transpose-back round.
  main (per 128-row tile, fully interleaved):
         32 fp32r matmuls -> PSUM blocks [128,2048]; diagonal self-sim killed
         with a -8*I add; DVE max8/find_index8 -> per-block top-8 candidates
         + positions; top-10 refine + softmax; pack gather indices; one
         dma_gather of proj[idx]; GPSIMD accumulates w*proj + residual; DMA
         out. Gather/apply of tile t overlaps the scans of tile t+1.
"""

from contextlib import ExitStack

import numpy as np

import concourse.bass as bass
import concourse.mybir as mybir
import concourse.tile as tile
from concourse import bacc
from concourse.bass_utils import run_bass_kernel_spmd
from concourse.masks import make_identity

FP32 = mybir.dt.float32
FP32R = mybir.dt.float32r
U16 = mybir.dt.uint16
I16 = mybir.dt.int16
I32 = mybir.dt.int32
AF = mybir.ActivationFunctionType
ALU = mybir.AluOpType
AX = mybir.AxisListType

N_NODES = 16384
D = 128
N_CORES = 8
K = 10
STRENGTH = 0.1
BLK = 2048  # sims column block (4 PSUM banks of fp32)
MMW = 512  # fp32 matmul moving-operand max
SENTINEL = -8.0


def build_nc(n_nodes=N_NODES, rows=N_NODES // N_CORES, n_cores=N_CORES,
             debug=False, stage=5, reps=1):
    nc = bacc.Bacc("TRN2", target_bir_lowering=False, debug=debug,
                   num_devices=n_cores)
    featT = nc.dram_tensor("featT", [D, n_nodes], FP32, kind="ExternalInput")
    feat_rows = nc.dram_tensor("feat_rows", [rows, D], FP32,
                               kind="ExternalInput")
    WT = nc.dram_tensor("WT", [D, D], FP32, kind="ExternalInput")
    emb = nc.dram_tensor("emb", [1, D], FP32, kind="ExternalInput")
    out_rows = nc.dram_tensor("out_rows", [rows, D], FP32,
                              kind="ExternalOutput")
    projdram = nc.dram_tensor("projdram", [n_nodes, D], FP32)
    invdram = nc.dram_tensor("invdram", [1, n_nodes], FP32)

    with tile.TileContext(nc) as tc, ExitStack() as ctx:
        _build(ctx, tc, featT.ap(), feat_rows.ap(), WT.ap(), emb.ap(),
               out_rows.ap(), projdram.ap(), invdram.ap(), n_nodes, rows,
               stage, reps)
    nc.compile()
    return nc


def _build(ctx, tc, featT, feat_rows, WT, emb, out_rows, projdram, invdram,
           n_nodes, rows, stage=5, reps=1):
    nc = tc.nc
    n_blocks = n_nodes // BLK          # 8
    n_tiles = rows // 128              # 16
    CW = n_blocks * 8                  # candidate width per row (64)

    consts = ctx.enter_context(tc.tile_pool(name="consts", bufs=1))
    bigbuf = ctx.enter_context(tc.tile_pool(name="bigbuf", bufs=1))
    stream = ctx.enter_context(tc.tile_pool(name="stream", bufs=2))
    small = ctx.enter_context(tc.tile_pool(name="small", bufs=3))
    tpool = ctx.enter_context(tc.tile_pool(name="tpool", bufs=4))
    gpool = ctx.enter_context(tc.tile_pool(name="gpool", bufs=8))
    psum_blk = ctx.enter_context(
        tc.tile_pool(name="psum_blk", bufs=2, space="PSUM"))

    # ---------------- constants ----------------
    ident = consts.tile([128, 128], FP32)
    make_identity(nc, ident[:])
    negI = consts.tile([128, 128], FP32)
    nc.gpsimd.memset(negI[:], 0.0)
    nc.gpsimd.affine_select(
        out=negI[:], in_=negI[:], compare_op=ALU.not_equal, fill=SENTINEL,
        base=0, pattern=[[-1, 128]], channel_multiplier=1)

    iota_i = consts.tile([128, CW], I32)
    nc.gpsimd.iota(iota_i[:], pattern=[[1, CW]], base=0, channel_multiplier=0)
    iotaF = consts.tile([128, CW], FP32)
    nc.vector.tensor_copy(iotaF[:], iota_i[:])

    bb_i = consts.tile([128, CW], I32)
    nc.gpsimd.iota(bb_i[:], pattern=[[BLK, n_blocks], [0, 8]], base=0,
                   channel_multiplier=0)
    bbF = consts.tile([128, CW], FP32)
    nc.vector.tensor_copy(bbF[:], bb_i[:])

    emb_bc = consts.tile([128, D], FP32)
    nc.sync.dma_start(out=emb_bc[:], in_=emb.to_broadcast([128, D]))
    emb01 = consts.tile([128, D], FP32)
    nc.scalar.mul(emb01[:], emb_bc[:], STRENGTH)

    WT_sb = consts.tile([D, D], FP32)
    nc.sync.dma_start(out=WT_sb[:], in_=WT)

    def _body():
        # ------- prep: projT -> pnat + norms -> inv-flat -> nrmT (in place) ---
        nrmT = bigbuf.tile([128, n_nodes], FP32)   # holds projT, then nrmT
        nchunks = n_nodes // 128              # 128
        ngrp = nchunks // 4                   # 512-wide groups (32)
        ssq_all = consts.tile([128, nchunks], FP32)

        for b in range(n_blocks):
            fT = stream.tile([128, BLK], FP32, tag="ftblk")
            nc.sync.dma_start(out=fT[:], in_=featT[:, b * BLK:(b + 1) * BLK])
            pA = psum_blk.tile([128, BLK], FP32, tag="blk")
            for m in range(BLK // MMW):
                nc.tensor.matmul(pA[:, m * MMW:(m + 1) * MMW], lhsT=WT_sb[:],
                                 rhs=fT[:, m * MMW:(m + 1) * MMW], start=True,
                                 stop=True)
            if b % 2 == 0:
                nc.scalar.copy(nrmT[:, b * BLK:(b + 1) * BLK], pA[:])
            else:
                nc.vector.tensor_copy(nrmT[:, b * BLK:(b + 1) * BLK], pA[:])

        sqscr = consts.tile([128, 512], FP32)
        for g in range(ngrp):
            pT = psum_blk.tile([128, BLK], FP32, tag="blk")
            pG = pT[:, 0:512]
            for j in range(4):
                c = 4 * g + j
                nc.tensor.transpose(pG[:, 128 * j:128 * (j + 1)],
                                    nrmT[:, c * 128:(c + 1) * 128], ident[:])
            pgrp = stream.tile([128, 512], FP32, tag="pgrp")
            nc.scalar.copy(pgrp[:], pG)
            sq = stream.tile([128, 512], FP32, tag="sqg")
            nc.scalar.activation(sq[:], pG, AF.Square)
            nc.vector.tensor_reduce(
                ssq_all[:, 4 * g:4 * (g + 1)],
                sq[:].rearrange("p (c d) -> p c d", d=128),
                axis=AX.X, op=ALU.add)
            # proj rows for this group -> DRAM (gather source), unnormalized
            nc.sync.dma_start(
                out=projdram[4 * g * 128:(4 * g + 4) * 128, :].rearrange(
                    "(c p) d -> p c d", p=128),
                in_=pgrp[:].rearrange("p (c d) -> p c d", d=128))
        # inv-norms, indexed by node: invT[c, p] = 1/nrm(node c*128+p)
        nrm_all = consts.tile([128, nchunks], FP32)
        nc.scalar.sqrt(nrm_all[:], ssq_all[:])
        inv_all = consts.tile([128, nchunks], FP32)
        nc.vector.reciprocal(inv_all[:], nrm_all[:])
        pT = psum_blk.tile([128, BLK], FP32, tag="blk")
        nc.tensor.transpose(pT[:, 0:128], inv_all[:], ident[:])
        invT = consts.tile([128, nchunks], FP32)
        nc.scalar.copy(invT[:], pT[:, 0:128])
        nc.sync.dma_start(
            out=invdram.rearrange("q (c p) -> q c p", p=128),
            in_=invT[:].rearrange("c (q p) -> c q p", q=1))
        inv_bc = bigbuf.tile([128, n_nodes], FP32)
        # column-scale projT in place -> nrmT (broadcast DMA split + interleaved)
        SC = 4096
        for s in range(n_nodes // SC):
            nc.sync.dma_start(
                out=inv_bc[:, s * SC:(s + 1) * SC],
                in_=invdram[:, s * SC:(s + 1) * SC].to_broadcast([128, SC]))
        for s in range(n_nodes // SC):
            nc.vector.scalar_tensor_tensor(
                out=nrmT[:, s * SC:(s + 1) * SC],
                in0=nrmT[:, s * SC:(s + 1) * SC], scalar=1.0,
                in1=inv_bc[:, s * SC:(s + 1) * SC],
                op0=ALU.mult, op1=ALU.mult)

        def _bail():
            for t in range(n_tiles):
                ft = small.tile([128, D], FP32, tag="ft")
                nc.sync.dma_start(out=ft[:],
                                  in_=feat_rows[t * 128:(t + 1) * 128, :])
                o = small.tile([128, D], FP32, tag="oo")
                nc.vector.tensor_add(o[:], ft[:], nrmT[:, t * 128:(t + 1) * 128])
                nc.sync.dma_start(out=out_rows[t * 128:(t + 1) * 128, :], in_=o[:])

        if stage <= 1:
            _bail()
            return

        # ------- main: per tile: sims + candidates + topk + gather launch;
        # the weighted apply of tile t-1 is issued after tile t's scans so the
        # gather DMA never stalls the DVE stream. -------
        def _front(t):
            lhs = nrmT[:, t * 128:(t + 1) * 128]
            cand = tpool.tile([128, CW], FP32, tag="cand")
            cidx = tpool.tile([128, CW], U16, tag="cidx")
            for b in range(n_blocks):
                ps = psum_blk.tile([128, BLK], FP32, tag="blk")
                for m in range(BLK // MMW):
                    nc.tensor.matmul(
                        ps[:, m * MMW:(m + 1) * MMW], lhsT=lhs,
                        rhs=nrmT[:, b * BLK + m * MMW:b * BLK + (m + 1) * MMW],
                        start=True, stop=True)
                if b == (t * 128) // BLK:
                    off = (t * 128) % BLK
                    nc.vector.tensor_add(ps[:, off:off + 128],
                                         ps[:, off:off + 128], negI[:])
                nc.vector.max(out=cand[:, b * 8:(b + 1) * 8], in_=ps[:])
                nc.vector.max_index(out=cidx[:, b * 8:(b + 1) * 8],
                                    in_max=cand[:, b * 8:(b + 1) * 8],
                                    in_values=ps[:])
            if stage <= 2:
                return None
            # global candidate columns (fp32 exact)
            cgid = tpool.tile([128, CW], FP32, tag="cgid")
            nc.scalar.copy(cgid[:], cidx[:])
            nc.vector.tensor_add(cgid[:], cgid[:], bbF[:])

            top8a = tpool.tile([128, 8], FP32, tag="t8a")
            nc.vector.max(out=top8a[:], in_=cand[:])
            cand2 = tpool.tile([128, CW], FP32, tag="cand2")
            nc.vector.match_replace(out=cand2[:], in_to_replace=top8a[:],
                                    in_values=cand[:], imm_value=SENTINEL)
            top8b = tpool.tile([128, 8], FP32, tag="t8b")
            nc.vector.max(out=top8b[:], in_=cand2[:])
            posa = tpool.tile([128, 8], U16, tag="posa")
            nc.vector.max_index(out=posa[:], in_max=top8a[:], in_values=cand[:])
            posb = tpool.tile([128, 8], U16, tag="posb")
            nc.vector.max_index(out=posb[:], in_max=top8b[:], in_values=cand2[:])

            P16 = tpool.tile([128, 16], FP32, tag="p16")
            nc.scalar.copy(P16[:, 0:8], posa[:])
            nc.scalar.copy(P16[:, 8:16], posb[:])

            # exp of top-10 values; Z; 0.1/Z
            V10 = tpool.tile([128, K], FP32, tag="v10")
            nc.scalar.copy(V10[:, 0:8], top8a[:])
            nc.scalar.copy(V10[:, 8:K], top8b[:, 0:K - 8])
            E10 = tpool.tile([128, K], FP32, tag="e10")
            nc.scalar.activation(E10[:], V10[:], AF.Exp)

            # per-row global columns of the top-10 (one-hot dot)
            gf = tpool.tile([128, K], FP32, tag="gf")
            for k in range(K):
                scr = small.tile([128, CW], FP32, tag="scr")
                nc.vector.scalar_tensor_tensor(
                    out=scr[:], in0=iotaF[:], scalar=P16[:, k:k + 1], in1=cgid[:],
                    op0=ALU.is_equal, op1=ALU.mult,
                    accum_out=gf[:, k:k + 1])
            gidx16 = tpool.tile([128, K], I16, tag="gidx")
            nc.vector.tensor_copy(gidx16[:], gf[:])
            if stage <= 3:
                return None

            # pack gather indices: idx i at [i % 16, i // 16] replicated
            # across all eight 16-partition groups; i = j*128 + p.
            idxw = tpool.tile([128, K * 8], I16, tag="idxw")
            for a in range(8):
                nc.sync.dma_start(
                    out=idxw[0:16, a:K * 8:8],
                    in_=gidx16[16 * a:16 * (a + 1), :])
            for q in range(1, 8):
                nc.sync.dma_start(out=idxw[16 * q:16 * (q + 1), :],
                                  in_=idxw[0:16, :])
            GCOLS = 5
            Gs = []
            for h in range(2):
                G = gpool.tile([128, GCOLS, D], FP32, tag="gath")
                nc.gpsimd.dma_gather(
                    out_ap=G[:], in_ap=projdram,
                    idxs_ap=idxw[:, h * GCOLS * 8:(h + 1) * GCOLS * 8],
                    num_idxs=GCOLS * 128, num_idxs_reg=GCOLS * 128,
                    elem_size=D, queue_num=0)
                Gs.append(G)
            if stage <= 4:
                return None
            return (t, Gs, E10)

        def _apply(state):
            t, Gs, E10 = state
            # softmax normalization deferred here: E10 (ACT) is long done, so
            # the refine's DVE stream never waits on the Scalar engine.
            Z = tpool.tile([128, 1], FP32, tag="zz")
            nc.vector.tensor_reduce(Z[:], E10[:], axis=AX.X, op=ALU.add)
            invZ01 = tpool.tile([128, 1], FP32, tag="iz")
            nc.vector.reciprocal(invZ01[:], Z[:])
            nc.vector.tensor_scalar_mul(invZ01[:], invZ01[:], STRENGTH)
            acc = tpool.tile([128, D], FP32, tag="acc")
            nc.vector.memset(acc[:], 0.0)
            for k in range(0, K):
                nc.vector.scalar_tensor_tensor(
                    out=acc[:], in0=Gs[k // 5][:, k % 5, :].squeeze(),
                    scalar=E10[:, k:k + 1], in1=acc[:],
                    op0=ALU.mult, op1=ALU.add)
            ft = tpool.tile([128, D], FP32, tag="ft")
            nc.sync.dma_start(out=ft[:],
                              in_=feat_rows[t * 128:(t + 1) * 128, :])
            o = tpool.tile([128, D], FP32, tag="oo")
            nc.vector.scalar_tensor_tensor(
                out=o[:], in0=acc[:], scalar=invZ01[:], in1=ft[:],
                op0=ALU.mult, op1=ALU.add)
            nc.vector.tensor_add(o[:], o[:], emb01[:])
            nc.sync.dma_start(out=out_rows[t * 128:(t + 1) * 128, :],
                              in_=o[:])

        from collections import deque
        pending = deque()
        for t in range(n_tiles):
            if stage <= 2 and t > 0:
                break
            st = _front(t)
            if st is not None:
                pending.append(st)
            while len(pending) > 2:
                _apply(pending.popleft())
        while pending:
            _apply(pending.popleft())

    if reps == 1:
        _body()
    else:
        with tc.For_i(0, reps, 1):
            _body()


_NC_CACHE = {}


def _get_nc(n_nodes, rows, n_cores):
    key = (n_nodes, rows, n_cores)
    if key not in _NC_CACHE:
        _NC_CACHE[key] = build_nc(n_nodes, rows, n_cores)
    return _NC_CACHE[key]


def make_in_maps(feat, W, emb, n_cores=N_CORES):
    n = feat.shape[0]
    rows = n // n_cores
    featT = np.ascontiguousarray(feat.T.astype(np.float32))
    WT = np.ascontiguousarray(W.T.astype(np.float32))
    emb = np.ascontiguousarray(emb.astype(np.float32))
    maps = []
    for c in range(n_cores):
        maps.append({
            "featT": np.ascontiguousarray(np.roll(featT, -rows * c, axis=1)),
            "feat_rows": np.ascontiguousarray(feat[rows * c:rows * (c + 1)]),
            "WT": WT,
            "emb": emb,
        })
    return maps


def kernel(feat, W, emb):
    feat = np.asarray(feat, dtype=np.float32)
    W = np.asarray(W, dtype=np.float32)
    emb = np.asarray(emb, dtype=np.float32)
    n = feat.shape[0]
    rows = n // N_CORES
    nc = _get_nc(n, rows, N_CORES)
    in_maps = make_in_maps(feat, W, emb, N_CORES)
    res = run_bass_kernel_spmd(nc, in_maps, core_ids=list(range(N_CORES)))
    out = np.concatenate([res.results[c]["out_rows"] for c in range(N_CORES)],
                         axis=0)
    return out.astype(np.float32)
